# revision 1
# baseline (speedup 1.0000x reference)
"""Trainium2 Bass kernel for nn_MetaFunUpdaterLocal (gnn_message_passing).

Math (per meta-batch b, per outer-tile k):
    h    = concat([x[b], y[b], r_c[b,k]], -1)           [C, 160]
    U    = MLP(h)  (160->128 relu ->128 relu ->64)      [C, 64]
    next_r_c[b,k] = r_c[b,k] - 0.1 * c_att[b] @ U
    next_r_q[b,k] = r_q[b,k] - 0.1 * q_att[b] @ U

Key restructurings:
  * The x/y part of layer 1 is k-independent: P[b] = [x|y]@W1[:96] + b1 is
    precomputed on host and injected into PSUM with an exact identity matmul.
  * Everything on-device runs in "feature-major" (transposed) space: the host
    ships rT[b,g] = [128, 512] tiles holding the TRANSPOSED r_c / r_q of two
    consecutive pairs (g = pair group), so the same SBUF tile is both the
    layer-1 matmul rhs (contraction dim d on partitions) and the update-term
    operand, with fully contiguous 256 KiB DMAs in and out.
  * All big matmuls run as float32r (full PE rate for moving dim >= 256,
    fp32 storage); only the tiny layer-3 (K=128 -> 64) runs in bf16.
  * delta_c / delta_q for the two pairs are computed with pair-packed
    stationary operands (M = 2x64 = 128) so the PE array is fully used, and
    the attention maps are pre-scaled by -0.1 so the final update is a single
    [128, 512] tensor_tensor add straight out of PSUM.

Sharding: 8 cores, core c handles b = c//2 and a 64-group (128-pair) slice of
the outer C axis (matches the sharding hint: B x outer-C data parallel).
"""

import numpy as np

B, C, Q, XD, YD, E, H = 4, 256, 256, 64, 32, 64, 128
NCORES = 8
NG_CORE = 64  # 2-pair groups per core (64 groups x 2 pairs x 8 cores = 1024 pairs)

_NC_CACHE = {}


def _build_nc(ng=NG_CORE, stage=5):
    import concourse.bass as bass
    import concourse.bacc as bacc
    import concourse.mybir as mybir
    import concourse.tile as tile
    from concourse.bass import _add_dep_helper

    F32 = mybir.dt.float32
    F32R = mybir.dt.float32r
    BF16 = mybir.dt.bfloat16
    ADD = mybir.AluOpType.add
    MAX = mybir.AluOpType.max
    RELU = mybir.ActivationFunctionType.Relu
    COPY = mybir.ActivationFunctionType.Copy

    # Bacc (not raw Bass): its finalize() runs move_matmul_waits_to_ldweights
    # + generate_event_semaphores, which split multi-sem waits to satisfy the
    # TRN2 one-wait-per-instruction constraint.
    nc = bacc.Bacc("TRN2", target_bir_lowering=False, debug=False,
                   num_devices=NCORES)

    # all fp32 constants ship in ONE dma (single wait lane for consumers):
    # cols: [0:512 pt2][512:1024 ac][1024:1536 aq][1536:1664 i128]
    #       [1664:1792 w1d2][1792:1920 w2][1920:1984 w3(f32)]
    #       row0: [1984:2112 ones][2112:2368 b3r]
    CB = 2368
    rt_d = nc.dram_tensor("rt", [ng, 128, 512], F32R, kind="ExternalInput")
    cb_d = nc.dram_tensor("cbig", [128, CB], F32R, kind="ExternalInput")
    out_d = nc.dram_tensor("out", [ng, 128, 512], F32, kind="ExternalOutput")

    with tile.TileContext(nc) as tc:
        with (
            tc.tile_pool(name="const", bufs=1) as cp,
            tc.tile_pool(name="rt", bufs=4) as rtp,
            tc.tile_pool(name="rtr", bufs=3) as rp2,
            tc.tile_pool(name="s", bufs=4) as sp,
            tc.tile_pool(name="u", bufs=3) as up,
            tc.tile_pool(name="o", bufs=4) as op,
            tc.tile_pool(name="pz", bufs=4, space="PSUM") as pz,
            tc.tile_pool(name="pu", bufs=2, space="PSUM") as pu,
            tc.tile_pool(name="pd", bufs=2, space="PSUM") as pd,
        ):
            cbig = cp.tile([128, CB], F32R)
            nc.sync.dma_start(cbig[:], cb_d[:, :])
            pt2 = cbig[:, 0:512]
            ac = cbig[:, 512:1024]
            aq = cbig[:, 1024:1536]
            i128 = cbig[:, 1536:1664]
            w1d2 = cbig[:, 1664:1792]
            w2 = cbig[:, 1792:1920]
            ones1 = cbig[0:1, 1984:2112]
            b3r = cbig[0:1, 2112:2368]
            # w3 via DVE cast-copy so layer-3 matmul deps stay on the DVE sem
            w3 = cp.tile([128, 64], BF16)
            nc.vector.tensor_copy(w3[:], cbig[:, 1920:1984].bitcast(F32))

            def chain(mms):
                for a, b_ in zip(mms[1:], mms):
                    _add_dep_helper(a.ins, b_.ins, sync=False, reason="psum order")

            for g in range(ng):
                rt = rtp.tile([128, 512], F32R)
                ld = nc.sync.dma_start(rt[:], rt_d[g, :, :])
                # DVE nop that owns the rt-DMA wait: the HW allows only ONE
                # sync-wait per compute instruction, so the final update op
                # must not need both the PE (dp) and DMA (rt) waits itself.
                nop = nc.vector.engine_nop()
                _add_dep_helper(nop.ins, ld.ins, sync=True,
                                reason="absorb rt dma wait on DVE")
                # rcT of both pairs side-by-side at base partition 0 (matmul
                # operands at base_partition 64 fault on HW): SBUF->SBUF remap
                rtr = rp2.tile([64, 512], F32R)
                nc.sync.dma_start(rtr[:, 0:256], rt[0:64, 0:256])
                nc.sync.dma_start(rtr[:, 256:512], rt[64:128, 0:256])

                o2 = op.tile([128, 512], F32)
                if stage == 0:
                    nc.vector.tensor_tensor(o2[:], rt[:].bitcast(F32),
                                            rt[:].bitcast(F32), op=ADD)
                    nc.sync.dma_start(out_d[g, :, :], o2[:])
                    continue

                # ---- layer 1: Z1[h, i] = P[b].T (+) W1d.T @ rcT, per pair
                z1 = pz.tile([128, 512], F32, tag="z")
                if stage == 11:  # identity matmul only (f32r)
                    nc.tensor.matmul(z1[:], i128[:], pt2[:], start=True, stop=True)
                else:
                    m0 = nc.tensor.matmul(z1[:], i128[:], pt2[:],
                                          start=True, stop=False)
                    m1 = nc.tensor.matmul(z1[:], w1d2[0:64, :], rtr[:],
                                          start=False, stop=True)
                    chain([m0, m1])
                s1 = sp.tile([128, 512], F32R, tag="s1")
                nc.scalar.activation(s1[:], z1[:], RELU)
                if stage in (1, 11):
                    nc.vector.tensor_copy(o2[:], s1[:].bitcast(F32))
                    nc.sync.dma_start(out_d[g, :, :], o2[:])
                    continue

                # ---- layer 2
                z2 = pz.tile([128, 512], F32, tag="z")
                nc.tensor.matmul(z2[:], w2[:], s1[:], start=True, stop=True)
                s2 = sp.tile([128, 512], BF16, tag="s2")
                nc.vector.tensor_scalar_max(s2[:], z2[:], 0.0)
                if stage == 2:
                    nc.vector.tensor_copy(o2[:], s2[:])
                    nc.sync.dma_start(out_d[g, :, :], o2[:])
                    continue

                # ---- layer 3: U[j, e] per (pair, j-chunk), +b3 via K=1 matmul
                # ups columns: [A-ch0 | B-ch0 | A-ch1 | B-ch1], 64 each
                ups = pu.tile([128, 256], F32)
                mb3 = nc.tensor.matmul(ups[:], ones1[:, :], b3r[:, :],
                                       start=True, stop=False)
                umms = [mb3]
                for ch in range(2):
                    for p in range(2):
                        mm = nc.tensor.matmul(
                            ups[:, ch * 128 + p * 64: ch * 128 + p * 64 + 64],
                            s2[:, p * 256 + ch * 128: p * 256 + (ch + 1) * 128],
                            w3[:],
                            start=False, stop=(ch == 1 and p == 1))
                        umms.append(mm)
                chain(umms)
                u = up.tile([128, 256], F32R)
                nc.scalar.activation(u[:], ups[:], COPY)
                if stage == 3:
                    nc.vector.tensor_copy(o2[:, 0:256], u[:].bitcast(F32))
                    nc.vector.tensor_copy(o2[:, 256:512], u[:].bitcast(F32))
                    nc.sync.dma_start(out_d[g, :, :], o2[:])
                    continue

                # ---- deltas: pd = [-0.1*Ac@U | -0.1*Aq@U], pair-packed M=128
                dp = pd.tile([128, 512], F32)
                d0 = nc.tensor.matmul(dp[:, 0:256], u[:, 0:128],
                                      ac[:, 0:256], start=True, stop=False)
                d1 = nc.tensor.matmul(dp[:, 0:256], u[:, 128:256],
                                      ac[:, 256:512], start=False, stop=False)
                d2 = nc.tensor.matmul(dp[:, 256:512], u[:, 0:128],
                                      aq[:, 0:256], start=False, stop=False)
                d3 = nc.tensor.matmul(dp[:, 256:512], u[:, 128:256],
                                      aq[:, 256:512], start=False, stop=True)
                chain([d0, d1, d2, d3])
                if stage == 4:
                    nc.vector.tensor_copy(o2[:], dp[:])
                    nc.sync.dma_start(out_d[g, :, :], o2[:])
                    continue

                # ---- update + store
                # first-writer memset absorbs the o2 slot-release (store DMA)
                # wait so the update op itself only waits on PE
                nc.vector.memset(o2[0:1, 0:1], 0.0)
                nc.vector.tensor_tensor(o2[:], rt[:].bitcast(F32), dp[:], op=ADD)
                nc.sync.dma_start(out_d[g, :, :], o2[:])

    nc.finalize()
    return nc


def _get_nc(ng=NG_CORE):
    if ng not in _NC_CACHE:
        _NC_CACHE[ng] = _build_nc(ng)
    return _NC_CACHE[ng]


def _host_prep(x, y, r_c, r_q, c_att_map, q_att_map, W1, b1, W2, b2, W3, b3):
    """Build per-core input maps. Returns (in_maps, meta)."""
    import ml_dtypes

    f32 = np.float32
    x = np.asarray(x, f32); y = np.asarray(y, f32)
    r_c = np.ascontiguousarray(np.asarray(r_c, f32))
    r_q = np.ascontiguousarray(np.asarray(r_q, f32))
    c_att = np.asarray(c_att_map, f32); q_att = np.asarray(q_att_map, f32)
    W1 = np.asarray(W1, f32); b1 = np.asarray(b1, f32)
    W2 = np.asarray(W2, f32); W3 = np.asarray(W3, f32); b3 = np.asarray(b3, f32)

    # P[b] = [x|y] @ W1[:96] + b1  (k-independent part of layer 1), transposed
    xy = np.concatenate([x, y], axis=-1)                      # [B, C, 96]
    P = xy @ W1[:XD + YD] + b1                                # [B, C, H]
    PT = np.ascontiguousarray(P.transpose(0, 2, 1))           # [B, H, C]
    pt2 = np.concatenate([PT, PT], axis=2)                    # [B, 128, 512]

    # rT[b, g] = [[rcT(2g); rcT(2g+1)] | [rqT(2g); rqT(2g+1)]]  -> [128, 512]
    rc2 = np.ascontiguousarray(r_c.transpose(0, 1, 3, 2)).reshape(B, C // 2, 128, 256)
    rq2 = np.ascontiguousarray(r_q.transpose(0, 1, 3, 2)).reshape(B, C // 2, 128, 256)
    rt = np.concatenate([rc2, rq2], axis=3)                   # [B, 128, 128, 512]

    # attention maps: transposed, chunked along j, pre-scaled by -ALPHA
    def att_chunks(a):  # [B, i, j] -> [B, 128, 512] = [-0.1*aT ch0 | ch1]
        at = (-0.1 * a.transpose(0, 2, 1)).astype(f32)        # [B, j, i]
        return np.ascontiguousarray(
            at.reshape(B, 2, 128, 256).transpose(0, 2, 1, 3)).reshape(B, 128, 512)

    ac = att_chunks(c_att)
    aq = att_chunks(q_att)

    w1d2 = np.concatenate([W1[XD + YD:], W1[XD + YD:]], axis=0)  # [128, 128]
    i128 = np.eye(128, dtype=f32)

    in_maps = []
    for core in range(NCORES):
        b = core // 2
        g0 = (core % 2) * NG_CORE
        cbig = np.zeros((128, 2368), f32)
        cbig[:, 0:512] = pt2[b]
        cbig[:, 512:1024] = ac[b]
        cbig[:, 1024:1536] = aq[b]
        cbig[:, 1536:1664] = i128
        cbig[:, 1664:1792] = w1d2
        cbig[:, 1792:1920] = W2
        cbig[:, 1920:1984] = W3
        cbig[0, 1984:2112] = 1.0
        cbig[0, 2112:2368] = np.tile(b3, 4)
        in_maps.append({
            "rt": rt[b, g0:g0 + NG_CORE],
            "cbig": cbig,
        })
    return in_maps


def _host_post(results):
    """results[core]["out"] [NG, 128, 512] -> (next_r_c, next_r_q) full."""
    next_r_c = np.empty((B, C, C, E), np.float32)
    next_r_q = np.empty((B, C, C, E), np.float32)
    for core in range(NCORES):
        out = results[core]["out"]                      # [64, 128, 512]
        b = core // 2
        k0 = (core % 2) * 128
        rc = out[:, :, 0:256].reshape(NG_CORE, 2, 64, 256)
        rq = out[:, :, 256:512].reshape(NG_CORE, 2, 64, 256)
        next_r_c[b, k0:k0 + 128] = rc.transpose(0, 1, 3, 2).reshape(128, 256, 64)
        next_r_q[b, k0:k0 + 128] = rq.transpose(0, 1, 3, 2).reshape(128, 256, 64)
    return next_r_c, next_r_q


def kernel(x, y, r_c, r_q, c_att_map, q_att_map, W1, b1, W2, b2, W3, b3,
           _trace=False, _trace_kwargs=None):
    import time
    from concourse.bass_utils import run_bass_kernel_spmd

    t0 = time.time()
    nc = _get_nc()
    t1 = time.time()
    in_maps = _host_prep(x, y, r_c, r_q, c_att_map, q_att_map,
                         W1, b1, W2, b2, W3, b3)
    t2 = time.time()
    res = run_bass_kernel_spmd(
        nc, in_maps, list(range(NCORES)),
        trace=_trace, **(_trace_kwargs or {}))
    t3 = time.time()
    out = _host_post(res.results)
    t4 = time.time()
    kernel.last_result = res
    kernel.timings = {"build": t1 - t0, "prep": t2 - t1, "run": t3 - t2,
                      "post": t4 - t3}
    return out



# revision 2
# speedup vs baseline: 2.1552x; 2.1552x over previous
"""Trainium2 Bass kernel for nn_MetaFunUpdaterLocal (gnn_message_passing).

Math (per meta-batch b, per outer-tile k):
    h    = concat([x[b], y[b], r_c[b,k]], -1)           [C, 160]
    U    = MLP(h)  (160->128 relu ->128 relu ->64)      [C, 64]
    next_r_c[b,k] = r_c[b,k] - 0.1 * c_att[b] @ U
    next_r_q[b,k] = r_q[b,k] - 0.1 * q_att[b] @ U

v2 restructurings (vs the fp32r baseline):
  * Everything runs in bf16 on the PE (fp32r matmuls execute in
    fp32_mode=HIGH on HW = 4 cycles/row; bf16 is 1). PSUM accum stays fp32.
    rel-err budget is 2e-2; bf16 end-to-end lands ~3e-3.
  * bf16 HBM I/O: r_c/r_q ship and return as bf16 -> half the DMA bytes.
  * The x/y part of layer 1 is k-independent: P[b] = [x|y]@W1[:96] + b1 is
    precomputed on host and injected into PSUM with an identity matmul.
  * Layer-1 uses BLOCK-DIAGONAL stationaries ([W1d;0] and [0;W1d]) so the
    pair-stacked rT tile feeds the matmul directly at base partition 0 --
    no SBUF->SBUF remap DMAs at all.
  * b3 enters only through the deltas as a rank-1 term; it is folded on the
    host (zero work when b3 == 0, which the spec guarantees).
  * Two pair-groups ship per DMA ("super tiles", [128, 1024] bf16 = 256 KiB,
    2 KiB per partition line).

Layouts (per pair group g = pairs A=2g, B=2g+1):
  rt[:, 0:256]  = [rcT_A ; rcT_B]   (e on partitions 0:64 / 64:128, i cols)
  rt[:, 256:512]= [rqT_A ; rqT_B]
  z1/s1/s2 [128, 512]: h on partitions, cols = (A i 0:256 | B i 256:512)
  ups/u [128, 256]: i-chunk on partitions, cols = [A0 e|B0 e|A1 e|B1 e]
  dp [128, 512]: [-0.1 dcT pack | -0.1 dqT pack], matches rt -> one DVE add.

Sharding: 8 cores, core c handles b = c//2 and a 128-pair slice of the
outer C axis (B x outer-C data parallel, per the sharding hint).
"""

import numpy as np

B, C, Q, XD, YD, E, H = 4, 256, 256, 64, 32, 64, 128
NCORES = 8
NG_CORE = 64   # 2-pair groups per core
NS_CORE = 32   # super tiles per core (2 groups each)

_NC_CACHE = {}

CB = 2112  # const cols (bf16): pt2 512 | ac 512 | aq 512 | i128 128
           #                    | w1A 128 | w1B 128 | w2 128 | w3 64


def _build_nc(ns=NS_CORE):
    import concourse.bass as bass
    import concourse.bacc as bacc
    import concourse.mybir as mybir
    import concourse.tile as tile
    from concourse.bass import _add_dep_helper

    F32 = mybir.dt.float32
    BF16 = mybir.dt.bfloat16
    ADD = mybir.AluOpType.add
    RELU = mybir.ActivationFunctionType.Relu

    nc = bacc.Bacc("TRN2", target_bir_lowering=False, debug=False,
                   num_devices=NCORES)

    rt_d = nc.dram_tensor("rt", [ns, 128, 1024], BF16, kind="ExternalInput")
    cb_d = nc.dram_tensor("cbig", [128, CB], BF16, kind="ExternalInput")
    out_d = nc.dram_tensor("out", [ns, 128, 1024], BF16, kind="ExternalOutput")

    with tile.TileContext(nc) as tc:
        with (
            tc.tile_pool(name="const", bufs=1) as cp,
            tc.tile_pool(name="rt", bufs=4) as rtp,
            tc.tile_pool(name="s1", bufs=3) as s1p,
            tc.tile_pool(name="s2", bufs=3) as s2p,
            tc.tile_pool(name="u", bufs=3) as up,
            tc.tile_pool(name="o", bufs=3) as op,
            tc.tile_pool(name="pz", bufs=4, space="PSUM") as pz,
            tc.tile_pool(name="pu", bufs=2, space="PSUM") as pu,
            tc.tile_pool(name="pd", bufs=2, space="PSUM") as pd,
        ):
            cbig = cp.tile([128, CB], BF16)
            nc.sync.dma_start(cbig[:], cb_d[:, :])
            pt2 = cbig[:, 0:512]
            ac = cbig[:, 512:1024]
            aq = cbig[:, 1024:1536]
            i128 = cbig[:, 1536:1664]
            w1A = cbig[:, 1664:1792]
            w1B = cbig[:, 1792:1920]
            w2 = cbig[:, 1920:2048]
            w3 = cbig[:, 2048:2112]

            def chain(mms):
                for a, b_ in zip(mms[1:], mms):
                    _add_dep_helper(a.ins, b_.ins, sync=False, reason="psum order")

            for s in range(ns):
                rt = rtp.tile([128, 1024], BF16)
                ld = nc.sync.dma_start(rt[:], rt_d[s, :, :])
                # DVE nop owns the rt-DMA wait: HW allows ONE sync-wait per
                # compute instruction; the final update op must only wait PE.
                nop = nc.vector.engine_nop()
                _add_dep_helper(nop.ins, ld.ins, sync=True,
                                reason="absorb rt dma wait on DVE")

                o2 = op.tile([128, 1024], BF16)
                # first-writer memset absorbs the o2 slot-release (store DMA)
                # wait so the update ops themselves only wait on PE
                nc.vector.memset(o2[0:1, 0:1], 0.0)

                for half in range(2):
                    c0 = half * 512
                    rts = rt[:, c0:c0 + 512]

                    # ---- layer 1: P inject + block-diag W1d, per pair
                    z1 = pz.tile([128, 512], F32, tag="z")
                    m0 = nc.tensor.matmul(z1[:], i128[:], pt2[:],
                                          start=True, stop=False)
                    m1 = nc.tensor.matmul(z1[:, 0:256], w1A[:], rts[:, 0:256],
                                          start=False, stop=False)
                    m2 = nc.tensor.matmul(z1[:, 256:512], w1B[:], rts[:, 0:256],
                                          start=False, stop=True)
                    chain([m0, m1, m2])
                    s1 = s1p.tile([128, 512], BF16, tag="s1")
                    nc.scalar.activation(s1[:], z1[:], RELU)

                    # ---- layer 2
                    z2 = pz.tile([128, 512], F32, tag="z")
                    nc.tensor.matmul(z2[:], w2[:], s1[:], start=True, stop=True)
                    s2 = s2p.tile([128, 512], BF16, tag="s2")
                    nc.scalar.activation(s2[:], z2[:], RELU)

                    # ---- layer 3: U[j, e] per (pair, j-chunk); b3 on host
                    ups = pu.tile([128, 256], F32)
                    u0 = nc.tensor.matmul(ups[:, 0:64], s2[:, 0:128],
                                          w3[:], start=True, stop=False)
                    u1 = nc.tensor.matmul(ups[:, 64:128], s2[:, 256:384],
                                          w3[:], start=False, stop=False)
                    u2 = nc.tensor.matmul(ups[:, 128:192], s2[:, 128:256],
                                          w3[:], start=False, stop=False)
                    u3 = nc.tensor.matmul(ups[:, 192:256], s2[:, 384:512],
                                          w3[:], start=False, stop=True)
                    chain([u0, u1, u2, u3])
                    u = up.tile([128, 256], BF16)
                    nc.vector.tensor_copy(u[:], ups[:])

                    # ---- deltas: dp = [-0.1*Ac@U | -0.1*Aq@U], pair-packed
                    dp = pd.tile([128, 512], F32)
                    d0 = nc.tensor.matmul(dp[:, 0:256], u[:, 0:128],
                                          ac[:, 0:256], start=True, stop=False)
                    d1 = nc.tensor.matmul(dp[:, 0:256], u[:, 128:256],
                                          ac[:, 256:512], start=False, stop=False)
                    d2 = nc.tensor.matmul(dp[:, 256:512], u[:, 0:128],
                                          aq[:, 0:256], start=False, stop=False)
                    d3 = nc.tensor.matmul(dp[:, 256:512], u[:, 128:256],
                                          aq[:, 256:512], start=False, stop=True)
                    chain([d0, d1, d2, d3])

                    # ---- update (bf16 + fp32 PSUM -> bf16)
                    nc.vector.tensor_tensor(o2[:, c0:c0 + 512], rts, dp[:],
                                            op=ADD)

                nc.sync.dma_start(out_d[s, :, :], o2[:])

    nc.finalize()
    return nc


def _get_nc(ns=NS_CORE):
    if ns not in _NC_CACHE:
        _NC_CACHE[ns] = _build_nc(ns)
    return _NC_CACHE[ns]


def _host_prep(x, y, r_c, r_q, c_att_map, q_att_map, W1, b1, W2, b2, W3, b3):
    """Build per-core input maps. Returns in_maps."""
    import ml_dtypes

    f32 = np.float32
    bf16 = ml_dtypes.bfloat16
    x = np.asarray(x, f32); y = np.asarray(y, f32)
    r_c = np.ascontiguousarray(np.asarray(r_c, f32))
    r_q = np.ascontiguousarray(np.asarray(r_q, f32))
    c_att = np.asarray(c_att_map, f32); q_att = np.asarray(q_att_map, f32)
    W1 = np.asarray(W1, f32); b1 = np.asarray(b1, f32)
    W2 = np.asarray(W2, f32); W3 = np.asarray(W3, f32)

    # P[b] = [x|y] @ W1[:96] + b1  (k-independent part of layer 1), transposed
    xy = np.concatenate([x, y], axis=-1)                      # [B, C, 96]
    P = xy @ W1[:XD + YD] + b1                                # [B, C, H]
    PT = P.transpose(0, 2, 1)                                 # [B, H, C]
    pt2 = np.concatenate([PT, PT], axis=2)                    # [B, 128, 512]

    # rT[b, g] = [[rcT(2g); rcT(2g+1)] | [rqT(2g); rqT(2g+1)]]  -> [128, 512]
    rc2 = np.ascontiguousarray(
        r_c.transpose(0, 1, 3, 2)).reshape(B, C // 2, 128, 256)
    rq2 = np.ascontiguousarray(
        r_q.transpose(0, 1, 3, 2)).reshape(B, C // 2, 128, 256)
    rt = np.concatenate([rc2, rq2], axis=3)                   # [B, 128, 128, 512]
    # super tiles: two groups side by side -> [B, 64, 128, 1024]
    rts = rt.reshape(B, 64, 2, 128, 512).transpose(0, 1, 3, 2, 4) \
            .reshape(B, 64, 128, 1024).astype(bf16)

    # attention maps: transposed, chunked along j, pre-scaled by -ALPHA
    def att_chunks(a):  # [B, i, j] -> [B, 128, 512] = [-0.1*aT ch0 | ch1]
        at = (-0.1 * a.transpose(0, 2, 1)).astype(f32)        # [B, j, i]
        return np.ascontiguousarray(
            at.reshape(B, 2, 128, 256).transpose(0, 2, 1, 3)).reshape(B, 128, 512)

    ac = att_chunks(c_att)
    aq = att_chunks(q_att)

    W1d = W1[XD + YD:]                                        # [64, 128]
    zero64 = np.zeros((64, H), f32)
    w1A = np.concatenate([W1d, zero64], axis=0)               # [128, 128]
    w1B = np.concatenate([zero64, W1d], axis=0)
    i128 = np.eye(128, dtype=f32)

    in_maps = []
    for core in range(NCORES):
        b = core // 2
        s0 = (core % 2) * NS_CORE
        cbig = np.zeros((128, CB), f32)
        cbig[:, 0:512] = pt2[b]
        cbig[:, 512:1024] = ac[b]
        cbig[:, 1024:1536] = aq[b]
        cbig[:, 1536:1664] = i128
        cbig[:, 1664:1792] = w1A
        cbig[:, 1792:1920] = w1B
        cbig[:, 1920:2048] = W2
        cbig[:, 2048:2112] = W3
        in_maps.append({
            "rt": rts[b, s0:s0 + NS_CORE],
            "cbig": cbig.astype(bf16),
        })
    return in_maps


def _host_post(results, c_att_map, q_att_map, b3):
    """results[core]["out"] [NS, 128, 1024] -> (next_r_c, next_r_q) full."""
    next_r_c = np.empty((B, C, C, E), np.float32)
    next_r_q = np.empty((B, C, C, E), np.float32)
    for core in range(NCORES):
        out = np.asarray(results[core]["out"], dtype=np.float32)
        out = out.reshape(NS_CORE, 128, 2, 512).transpose(0, 2, 1, 3) \
                 .reshape(NG_CORE, 128, 512)                  # [64, 128, 512]
        b = core // 2
        k0 = (core % 2) * 128
        rc = out[:, :, 0:256].reshape(NG_CORE, 2, 64, 256)
        rq = out[:, :, 256:512].reshape(NG_CORE, 2, 64, 256)
        next_r_c[b, k0:k0 + 128] = rc.transpose(0, 1, 3, 2).reshape(128, 256, 64)
        next_r_q[b, k0:k0 + 128] = rq.transpose(0, 1, 3, 2).reshape(128, 256, 64)
    b3 = np.asarray(b3, np.float32)
    if np.any(b3):
        # rank-1 b3 term of the deltas, folded here: -0.1 * rowsum(att) x b3
        s_c = np.asarray(c_att_map, np.float32).sum(axis=2)   # [B, C]
        s_q = np.asarray(q_att_map, np.float32).sum(axis=2)   # [B, Q]
        next_r_c -= 0.1 * s_c[:, None, :, None] * b3[None, None, None, :]
        next_r_q -= 0.1 * s_q[:, None, :, None] * b3[None, None, None, :]
    return next_r_c, next_r_q


def kernel(x, y, r_c, r_q, c_att_map, q_att_map, W1, b1, W2, b2, W3, b3,
           _trace=False, _trace_kwargs=None):
    import time
    from concourse.bass_utils import run_bass_kernel_spmd

    t0 = time.time()
    nc = _get_nc()
    t1 = time.time()
    in_maps = _host_prep(x, y, r_c, r_q, c_att_map, q_att_map,
                         W1, b1, W2, b2, W3, b3)
    t2 = time.time()
    res = run_bass_kernel_spmd(
        nc, in_maps, list(range(NCORES)),
        trace=_trace, **(_trace_kwargs or {}))
    t3 = time.time()
    out = _host_post(res.results, c_att_map, q_att_map, b3)
    t4 = time.time()
    kernel.last_result = res
    kernel.timings = {"build": t1 - t0, "prep": t2 - t1, "run": t3 - t2,
                      "post": t4 - t3}
    return out


# revision 15
# speedup vs baseline: 2.5220x; 1.1702x over previous
"""Trainium2 Bass kernel for nn_MetaFunUpdaterLocal (gnn_message_passing).

Math (per meta-batch b, per outer-tile k):
    h    = concat([x[b], y[b], r_c[b,k]], -1)           [C, 160]
    U    = MLP(h)  (160->128 relu ->128 relu ->64)      [C, 64]
    next_r_c[b,k] = r_c[b,k] - 0.1 * c_att[b] @ U
    next_r_q[b,k] = r_q[b,k] - 0.1 * q_att[b] @ U

v4 structure (one "super" = 2 pair-groups = 4 pairs, [128, 1024] tiles):
  * All matmuls bf16 (fp32r executes in fp32_mode=HIGH = 4 cyc/row on HW),
    PSUM accum fp32, HBM I/O bf16. rel-err budget 2e-2; lands ~4e-3.
  * P[b] = [x|y]@W1[:96] + b1 precomputed on host, injected into PSUM with
    one identity matmul per super (N=1024).
  * Layer-1 uses BLOCK-DIAGONAL stationaries ([W1d;0], [0;W1d]) so the
    pair-stacked rT tile feeds matmuls at base partition 0 (no remaps).
  * Element-wise passes run at super granularity: one ACT relu for s1, one
    for s2, one DVE add for the update -- amortizes per-op init.
  * Deltas: one fp8e4 DoubleRow matmul per group (K = 2x128 j-positions,
    planes = j-chunks); b3's rank-1 delta term is folded on the host.
  * ups (layer-3 PSUM) lives in the SAME bank as dp: L3 writes it, the fp8
    cast reads it, then the DoubleRow matmul start=True re-zeroes the bank.
    PSUM = 2x z-super (4 banks) + 2x dp-super (4 banks) = all 8 banks.

Layouts (pair group g = pairs A=2g, B=2g+1; super s = groups 2s, 2s+1):
  rt [128, 2, 512]: [:, g, 0:256] = [rcT_A ; rcT_B], [:, g, 256:512] = rqT
  z1/s1/s2 [128, 1024]: h on partitions, cols = (g0 A i | g0 B i | g1 ...)
  dpS [128, 2, 512]: [:, g, :] = [-0.1 dcT pack | -0.1 dqT pack]
  u8 [128, 2, 128]: plane ch, cols [A-ch e | B-ch e] (DoubleRow stationary)

Sharding: 8 cores, core c handles b = c//2 and a 128-pair slice of the
outer C axis (B x outer-C data parallel, per the sharding hint).
"""

import numpy as np

B, C, Q, XD, YD, E, H = 4, 256, 256, 64, 32, 64, 128
NCORES = 8
NG_CORE = 64   # 2-pair groups per core
NS_CORE = 32   # super tiles per core (2 groups each)

_NC_CACHE = {}

CB = 1600  # const cols (bf16): pt4 1024 | i128 128 | w1A 128 | w1B 128
           #                    | w2 128 | w3 64


def _build_nc(ns=NS_CORE):
    import concourse.bass as bass
    import concourse.bacc as bacc
    import concourse.mybir as mybir
    import concourse.tile as tile
    from concourse.bass import _add_dep_helper

    F32 = mybir.dt.float32
    BF16 = mybir.dt.bfloat16
    FP8 = mybir.dt.float8e4
    DR = mybir.MatmulPerfMode.DoubleRow
    ADD = mybir.AluOpType.add
    RELU = mybir.ActivationFunctionType.Relu

    nc = bacc.Bacc("TRN2", target_bir_lowering=False, debug=False,
                   num_devices=NCORES)

    rt_d = nc.dram_tensor("rt", [ns, 128, 2, 512], BF16, kind="ExternalInput")
    cb_d = nc.dram_tensor("cbig", [128, CB], BF16, kind="ExternalInput")
    a8_d = nc.dram_tensor("a8", [128, 2, 512], FP8, kind="ExternalInput")
    out_d = nc.dram_tensor("out", [ns, 128, 1024], BF16, kind="ExternalOutput")

    with tile.TileContext(nc) as tc:
        with (
            tc.tile_pool(name="const", bufs=1) as cp,
            tc.tile_pool(name="rt", bufs=4) as rtp,
            tc.tile_pool(name="s1", bufs=2) as s1p,
            tc.tile_pool(name="s2", bufs=2) as s2p,
            tc.tile_pool(name="u", bufs=4) as up,
            tc.tile_pool(name="o", bufs=3) as op,
            tc.tile_pool(name="pz", bufs=2, space="PSUM") as pz,
            tc.tile_pool(name="pd", bufs=2, space="PSUM") as pd,
        ):
            cbig = cp.tile([128, CB], BF16)
            nc.sync.dma_start(cbig[:], cb_d[:, :])
            pt4 = cbig[:, 0:1024]
            i128 = cbig[:, 1024:1152]
            w1A = cbig[:, 1152:1280]
            w1B = cbig[:, 1280:1408]
            w2 = cbig[:, 1408:1536]
            w3 = cbig[:, 1536:1600]
            a8 = cp.tile([128, 2, 512], FP8)
            nc.sync.dma_start(a8[:], a8_d[:, :, :])

            def chain(mms):
                for a, b_ in zip(mms[1:], mms):
                    _add_dep_helper(a.ins, b_.ins, sync=False, reason="psum order")

            for s in range(ns):
                rt = rtp.tile([128, 2, 512], BF16)
                ld = nc.sync.dma_start(rt[:], rt_d[s, :, :, :])
                # DVE nop owns the rt-DMA wait: HW allows ONE sync-wait per
                # compute instruction; the update op must only wait PE.
                nop = nc.vector.engine_nop()
                _add_dep_helper(nop.ins, ld.ins, sync=True,
                                reason="absorb rt dma wait on DVE")

                o2 = op.tile([128, 1024], BF16)
                # first-writer memset absorbs the o2 slot-release (store DMA)
                # wait so the update op itself only waits on PE
                nc.vector.memset(o2[0:1, 0:1], 0.0)

                # ---- layer 1 (both groups): P inject + block-diag W1d
                # (each matmul must stay inside one PSUM bank: 512 f32 cols)
                z1 = pz.tile([128, 1024], F32, tag="z")
                ms = []
                for g in range(2):
                    c0 = g * 512
                    rc = rt[:, g, 0:256]
                    ms.append(nc.tensor.matmul(z1[:, c0:c0 + 512], i128[:],
                                               pt4[:, c0:c0 + 512],
                                               start=True, stop=False))
                    ms.append(nc.tensor.matmul(z1[:, c0:c0 + 256], w1A[:], rc,
                                               start=False, stop=False))
                    ms.append(nc.tensor.matmul(z1[:, c0 + 256:c0 + 512], w1B[:],
                                               rc, start=False, stop=True))
                chain(ms)
                s1 = s1p.tile([128, 1024], BF16, tag="s1")
                nc.scalar.activation(s1[:], z1[:], RELU)

                # ---- layer 2 (two matmuls, one per PSUM bank)
                z2 = pz.tile([128, 1024], F32, tag="z")
                l2a = nc.tensor.matmul(z2[:, 0:512], w2[:], s1[:, 0:512],
                                       start=True, stop=True)
                l2b = nc.tensor.matmul(z2[:, 512:1024], w2[:], s1[:, 512:1024],
                                       start=True, stop=True)
                chain([l2a, l2b])
                s2 = s2p.tile([128, 1024], BF16, tag="s2")
                nc.scalar.activation(s2[:], z2[:], RELU)

                # ---- layer 3 + deltas, per group, sharing the dp banks
                dpS = pd.tile([128, 2, 512], F32)
                for g in range(2):
                    b0 = g * 512
                    # U[j, e] tiles written into the front of dp's bank
                    um = [
                        nc.tensor.matmul(dpS[:, g, 0:64],
                                         s2[:, b0:b0 + 128], w3[:],
                                         start=True, stop=False),
                        nc.tensor.matmul(dpS[:, g, 64:128],
                                         s2[:, b0 + 256:b0 + 384], w3[:],
                                         start=False, stop=False),
                        nc.tensor.matmul(dpS[:, g, 128:192],
                                         s2[:, b0 + 128:b0 + 256], w3[:],
                                         start=False, stop=False),
                        nc.tensor.matmul(dpS[:, g, 192:256],
                                         s2[:, b0 + 384:b0 + 512], w3[:],
                                         start=False, stop=True),
                    ]
                    chain(um)
                    u8 = up.tile([128, 2, 128], FP8)
                    nc.vector.tensor_copy(u8[:], dpS[:, g, 0:256])
                    # one fp8 DoubleRow matmul: start=True re-zeroes the bank
                    # (ups is dead once the cast has read it)
                    dm = nc.tensor.matmul(dpS[:, g, :], u8[:], a8[:],
                                          start=True, stop=True, perf_mode=DR)
                    chain([um[-1], dm])

                # ---- update (one DVE add for the whole super)
                nc.vector.tensor_tensor(o2[:], rt[:], dpS[:], op=ADD)
                nc.sync.dma_start(out_d[s, :, :], o2[:])

    nc.finalize()
    return nc


def _get_nc(ns=NS_CORE):
    if ns not in _NC_CACHE:
        _NC_CACHE[ns] = _build_nc(ns)
    return _NC_CACHE[ns]


def _host_prep(x, y, r_c, r_q, c_att_map, q_att_map, W1, b1, W2, b2, W3, b3):
    """Build per-core input maps. Returns in_maps."""
    import ml_dtypes

    f32 = np.float32
    bf16 = ml_dtypes.bfloat16
    fp8 = ml_dtypes.float8_e4m3
    x = np.asarray(x, f32); y = np.asarray(y, f32)
    r_c = np.ascontiguousarray(np.asarray(r_c, f32))
    r_q = np.ascontiguousarray(np.asarray(r_q, f32))
    c_att = np.asarray(c_att_map, f32); q_att = np.asarray(q_att_map, f32)
    W1 = np.asarray(W1, f32); b1 = np.asarray(b1, f32)
    W2 = np.asarray(W2, f32); W3 = np.asarray(W3, f32)

    # P[b] = [x|y] @ W1[:96] + b1  (k-independent part of layer 1), transposed
    xy = np.concatenate([x, y], axis=-1)                      # [B, C, 96]
    P = xy @ W1[:XD + YD] + b1                                # [B, C, H]
    PT = P.transpose(0, 2, 1)                                 # [B, H, C]
    pt4 = np.concatenate([PT, PT, PT, PT], axis=2)            # [B, 128, 1024]

    # rT[b, g] = [[rcT(2g); rcT(2g+1)] | [rqT(2g); rqT(2g+1)]]  -> [128, 512]
    rc2 = np.ascontiguousarray(
        r_c.transpose(0, 1, 3, 2)).reshape(B, C // 2, 128, 256)
    rq2 = np.ascontiguousarray(
        r_q.transpose(0, 1, 3, 2)).reshape(B, C // 2, 128, 256)
    rt = np.concatenate([rc2, rq2], axis=3)                   # [B, 128, 128, 512]
    # super tiles: two groups each -> [B, 64, 128, 2, 512]
    rts = rt.reshape(B, 64, 2, 128, 512).transpose(0, 1, 3, 2, 4).astype(bf16)

    # attention maps: transposed, chunked along j, pre-scaled by -ALPHA
    def att_chunks(a):  # [B, i, j] -> [B, 128, 512] = [-0.1*aT ch0 | ch1]
        at = (-0.1 * a.transpose(0, 2, 1)).astype(f32)        # [B, j, i]
        return np.ascontiguousarray(
            at.reshape(B, 2, 128, 256).transpose(0, 2, 1, 3)).reshape(B, 128, 512)

    ac = att_chunks(c_att)
    aq = att_chunks(q_att)
    # fp8 DoubleRow moving operand: plane ch = [ac_ch | aq_ch]
    a8 = np.empty((B, 128, 2, 512), f32)
    a8[:, :, 0, 0:256] = ac[:, :, 0:256]
    a8[:, :, 0, 256:512] = aq[:, :, 0:256]
    a8[:, :, 1, 0:256] = ac[:, :, 256:512]
    a8[:, :, 1, 256:512] = aq[:, :, 256:512]
    a8 = a8.astype(fp8)

    W1d = W1[XD + YD:]                                        # [64, 128]
    zero64 = np.zeros((64, H), f32)
    w1A = np.concatenate([W1d, zero64], axis=0)               # [128, 128]
    w1B = np.concatenate([zero64, W1d], axis=0)
    i128 = np.eye(128, dtype=f32)

    in_maps = []
    for core in range(NCORES):
        b = core // 2
        s0 = (core % 2) * NS_CORE
        cbig = np.zeros((128, CB), f32)
        cbig[:, 0:1024] = pt4[b]
        cbig[:, 1024:1152] = i128
        cbig[:, 1152:1280] = w1A
        cbig[:, 1280:1408] = w1B
        cbig[:, 1408:1536] = W2
        cbig[:, 1536:1600] = W3
        in_maps.append({
            "rt": rts[b, s0:s0 + NS_CORE],
            "cbig": cbig.astype(bf16),
            "a8": a8[b],
        })
    return in_maps


def _host_post(results, c_att_map, q_att_map, b3):
    """results[core]["out"] [NS, 128, 1024] -> (next_r_c, next_r_q) full."""
    next_r_c = np.empty((B, C, C, E), np.float32)
    next_r_q = np.empty((B, C, C, E), np.float32)
    for core in range(NCORES):
        out = np.asarray(results[core]["out"], dtype=np.float32)
        out = out.reshape(NS_CORE, 128, 2, 512).transpose(0, 2, 1, 3) \
                 .reshape(NG_CORE, 128, 512)                  # [64, 128, 512]
        b = core // 2
        k0 = (core % 2) * 128
        rc = out[:, :, 0:256].reshape(NG_CORE, 2, 64, 256)
        rq = out[:, :, 256:512].reshape(NG_CORE, 2, 64, 256)
        next_r_c[b, k0:k0 + 128] = rc.transpose(0, 1, 3, 2).reshape(128, 256, 64)
        next_r_q[b, k0:k0 + 128] = rq.transpose(0, 1, 3, 2).reshape(128, 256, 64)
    b3 = np.asarray(b3, np.float32)
    if np.any(b3):
        # rank-1 b3 term of the deltas, folded here: -0.1 * rowsum(att) x b3
        s_c = np.asarray(c_att_map, np.float32).sum(axis=2)   # [B, C]
        s_q = np.asarray(q_att_map, np.float32).sum(axis=2)   # [B, Q]
        next_r_c -= 0.1 * s_c[:, None, :, None] * b3[None, None, None, :]
        next_r_q -= 0.1 * s_q[:, None, :, None] * b3[None, None, None, :]
    return next_r_c, next_r_q


def kernel(x, y, r_c, r_q, c_att_map, q_att_map, W1, b1, W2, b2, W3, b3,
           _trace=False, _trace_kwargs=None):
    import time
    from concourse.bass_utils import run_bass_kernel_spmd

    t0 = time.time()
    nc = _get_nc()
    t1 = time.time()
    in_maps = _host_prep(x, y, r_c, r_q, c_att_map, q_att_map,
                         W1, b1, W2, b2, W3, b3)
    t2 = time.time()
    res = run_bass_kernel_spmd(
        nc, in_maps, list(range(NCORES)),
        trace=_trace, **(_trace_kwargs or {}))
    t3 = time.time()
    out = _host_post(res.results, c_att_map, q_att_map, b3)
    t4 = time.time()
    kernel.last_result = res
    kernel.timings = {"build": t1 - t0, "prep": t2 - t1, "run": t3 - t2,
                      "post": t4 - t3}
    return out
